# revision 15
# baseline (speedup 1.0000x reference)
"""CAM-module (channel attention) Trainium2 kernel — fp16 single-pass version.

Problem: B=4 samples, C=64, H=W=256 (N=65536 px). concat(rgb,hsv,lab) ->
X [192, N] per sample; q/k/v = 1x1-conv projections (W [64,192] + bias);
energy = q @ k^T * C^-0.5 -> softmax over last dim -> out = att @ v.

Sharding: 8 cores = 4 samples x 2 spatial halves (32768 px each). Each
core computes a partial energy over its half; a 16 KiB pairwise AllReduce
([[0,1],[2,3],[4,5],[6,7]]) completes the C x C energy, then each core
computes out for its own half.

Precision: rel-err budget is 2e-2; numpy emulation of this exact scheme
measures ~1.6e-3. X and W are cast to fp16 host-side (RNE, identical to a
device cast); projections are single-pass fp16 matmuls with fp32 PSUM
accumulate. The dominant coherent error — the fixed fp16 weight truncation
dW hitting the Gram matrix G ~= N*I — is corrected host-side:
C = N*(dWq^T Wk16 + Wq16^T dWk + dWq^T dWk) is added to the energy after
the AllReduce. Energy runs as fp16 matmuls (1 cyc/row vs fp32's 4) on the
fp16-cast q/k tiles; residual rounding is incoherent (~0.02 scaled logits),
harmless against typical top-2 logit gaps (~400). The output returns as
fp16 (one more 2^-11 rounding) and is upcast host-side.

Performance structure (TRN2 cost model):
 - fp16 inputs halve the input stream to 12 MiB/core; out fp16 = 2 MiB.
 - rgb+hsv host-packed into one [128, NHALF] tensor -> 1 input DMA/tile.
 - v and out chunks are packed two-per-PSUM-tile at partition offsets 0/64,
   halving PSUM->SBUF copies; out leaves in a [128, NHALF/2] fp16 layout
   the host unpacks. PSUM->SBUF copies alternate ACT/DVE.
 - qk PSUM groups span 2 banks ([128,1024]) -> one fp16 cast per 8 subtiles.
 - PE/tile(2048px): proj 2x128x16 + energy 64x16 + v 4x512 = 9216 cyc
   (~3.8us warm) -> the main loop is PE-bound over a ~2.2us/tile DMA floor.
 - v is deferred 8 tiles so ~13us of v matmuls overlap the AllReduce.
"""

import sys
import numpy as np

if '/opt/trn_rl_repo' not in sys.path:
    sys.path.insert(0, '/opt/trn_rl_repo')

B, C, H, W = 4, 64, 256, 256
N = H * W                 # 65536 px per sample
NHALF = N // 2            # 32768 px per core
PX = 2048                 # streaming tile (px)
NIT = NHALF // PX         # 16
SUB = 128                 # proj subtile (px) = matmul M
NSUB = PX // SUB          # 16
VC = 512                  # v / out chunk (px) = matmul N
NVC = PX // VC            # 4
NCORES = 8
DEFER = 8                 # v-tiles deferred into the AllReduce window

_CACHE = {}


def _build_bass(single_core=False):
    import concourse.bacc as bacc
    import concourse.mybir as mybir
    from concourse import tile

    F32 = mybir.dt.float32
    F16 = mybir.dt.float16
    Exp = mybir.ActivationFunctionType.Exp

    nc = bacc.Bacc("TRN2", target_bir_lowering=False, debug=False,
                   enable_asserts=False,
                   num_devices=1 if single_core else NCORES)

    x01_d = nc.dram_tensor("x01", [128, NHALF], F16, kind="ExternalInput").ap()
    xb_d = nc.dram_tensor("x_lab", [64, NHALF], F16, kind="ExternalInput").ap()
    # packed fp16 weights: cols [wqk 0:128 | wv 128:192]
    w0_d = nc.dram_tensor("w0", [128, 192], F16, kind="ExternalInput").ap()
    w1_d = nc.dram_tensor("w1", [65, 192], F16, kind="ExternalInput").ap()
    ident_d = nc.dram_tensor("ident", [64, 64], F32, kind="ExternalInput").ap()
    corr_d = nc.dram_tensor("corr", [64, 64], F32, kind="ExternalInput").ap()
    # out, partition-pair packed: part p<64 = ch p, even 512-px chunk of the
    # 1024-col block; part p>=64 = ch p-64, odd chunk (host unpacks)
    out_d = nc.dram_tensor("out", [128, NHALF // 2], F16,
                           kind="ExternalOutput").ap()

    with tile.TileContext(nc) as tc:
        with tc.tile_pool(name="const", bufs=1) as const, \
             tc.tile_pool(name="s16", bufs=DEFER + 1) as s16, \
             tc.tile_pool(name="qk", bufs=4) as qkpool, \
             tc.tile_pool(name="outp", bufs=4) as outp, \
             tc.tile_pool(name="qkps", bufs=2, space="PSUM") as qkps, \
             tc.tile_pool(name="vps", bufs=2, space="PSUM") as vps, \
             tc.tile_pool(name="eps", bufs=1, space="PSUM") as eps, \
             tc.tile_pool(name="dram", bufs=1, space="DRAM") as dram:

            w0 = const.tile([128, 192], F16)
            w1 = const.tile([65, 192], F16)
            ident = const.tile([64, 64], F32)
            corr = const.tile([64, 64], F32)
            nc.scalar.dma_start(w0[:], w0_d[:])
            nc.scalar.dma_start(w1[:], w1_d[:])
            nc.scalar.dma_start(ident[:], ident_d[:])
            nc.scalar.dma_start(corr[:], corr_d[:])
            wqk0, wv0 = w0[:, 0:128], w0[:, 128:192]
            wqk1, wv1 = w1[:, 0:128], w1[:, 128:192]

            # preload the ACT Exp table set off the critical path (~2.7us)
            warm = const.tile([1, 1], F32)
            nc.gpsimd.memset(warm[:], 0.0)
            nc.scalar.activation(warm[:], warm[:], Exp)

            # v, partition-pair packed like out_d (4 MiB fp16)
            v16 = const.tile([128, NHALF // 2], F16)
            # paired energy accumulator: subtile pairs (2s, 2s+1) matmul as
            # lhsT=[q_2s|q_2s+1], rhs=[k_2s|k_2s+1]; the diagonal 64x64
            # blocks accumulate the true energy, off-diagonal is discarded
            ep = eps.tile([128, 128], F32)

            def v_block(vit, vx0h, vx1h):
                for pr in range(NVC // 2):        # chunk pairs
                    vp = vps.tile([128, VC], F32, tag="vp")
                    for h in range(2):
                        vsl = slice((2 * pr + h) * VC, (2 * pr + h + 1) * VC)
                        psl = slice(h * 64, h * 64 + 64)
                        nc.tensor.matmul(vp[psl, :], wv0[:], vx0h[:, vsl],
                                         start=True, stop=False)
                        nc.tensor.matmul(vp[psl, :], wv1[:], vx1h[:, vsl],
                                         start=False, stop=True)
                    blk = vit * (PX // 2) + pr * VC
                    if pr % 2 == 0:
                        nc.scalar.copy(v16[:, blk:blk + VC], vp[:])
                    else:
                        nc.vector.tensor_copy(v16[:, blk:blk + VC], vp[:])

            pending = []
            for it in range(NIT):
                sl = slice(it * PX, (it + 1) * PX)
                x0h = s16.tile([128, PX], F16, tag="x0h")
                nc.sync.dma_start(x0h[:], x01_d[:, sl])
                x1h = s16.tile([65, PX], F16, tag="x1h")
                nc.sync.dma_start(x1h[0:64, :], xb_d[:, sl])
                if it < DEFER + 1:
                    # ones rows live in the round-robin pool slots; later
                    # iterations reuse them untouched
                    nc.gpsimd.memset(x1h[64:65, :], 1.0)

                for grp in range(NSUB // 8):   # 8 proj subtiles per PSUM pair
                    qkp = qkps.tile([128, 1024], F32, tag="qkp")
                    for s8 in range(8):
                        sb = grp * 8 + s8
                        ssl = slice(sb * SUB, (sb + 1) * SUB)
                        osl = slice(s8 * 128, (s8 + 1) * 128)
                        nc.tensor.matmul(qkp[:, osl], x0h[:, ssl], wqk0[:],
                                         start=True, stop=False)
                        nc.tensor.matmul(qkp[:, osl], x1h[:, ssl], wqk1[:],
                                         start=False, stop=True)
                    # scatter-cast: qkp [s][q|k] -> qk_sb [q0..q7 | k0..k7]
                    # so energy subtile-pair operands are contiguous
                    qk_sb = qkpool.tile([128, 1024], F16, tag="qk_sb")
                    dst = qk_sb[:].rearrange("p (qk s b) -> p s qk b",
                                             qk=2, s=8, b=64)
                    srcv = qkp[:].rearrange("p (s qk b) -> p s qk b",
                                            s=8, qk=2, b=64)
                    if grp == 0:
                        nc.scalar.copy(dst, srcv)
                    else:
                        nc.vector.tensor_copy(dst, srcv)
                    for s4 in range(4):
                        first = (it == 0 and grp == 0 and s4 == 0)
                        last = (it == NIT - 1 and grp == NSUB // 8 - 1 and s4 == 3)
                        nc.tensor.matmul(ep[:], qk_sb[:, s4 * 128:s4 * 128 + 128],
                                         qk_sb[:, 512 + s4 * 128:512 + s4 * 128 + 128],
                                         start=first, stop=last)

                if it >= DEFER:
                    v_block(it - DEFER, *pending.pop(0))
                pending.append((x0h, x1h))

            # partial energy -> pairwise AllReduce; the deferred v matmuls
            # below keep PE busy while the collective is in flight
            ep_sb = const.tile([128, 128], F32)
            nc.scalar.copy(ep_sb[:], ep[:])
            ebr = const.tile([64, 64], F32)
            nc.sync.dma_start(ebr[:], ep_sb[64:128, 64:128])
            e_sb = const.tile([64, 64], F32)
            nc.vector.tensor_add(e_sb[:], ep_sb[0:64, 0:64], ebr[:])
            bi = dram.tile([64, 64], F32)
            bo = dram.tile([64, 64], F32)
            nc.sync.dma_start(bi[:], e_sb[:])
            if single_core:
                nc.gpsimd.dma_start(bo[:], bi[:])
            else:
                nc.gpsimd.collective_compute(
                    "AllReduce", mybir.AluOpType.add,
                    replica_groups=[[0, 1], [2, 3], [4, 5], [6, 7]],
                    ins=[bi.opt()], outs=[bo.opt()],
                )

            for d in range(DEFER):
                v_block(NIT - DEFER + d, *pending[d])

            e2 = const.tile([64, 64], F32)
            nc.sync.dma_start(e2[:], bo[:])
            # fp16-weight-truncation correction (see module docstring)
            e2c = const.tile([64, 64], F32)
            nc.vector.tensor_add(e2c[:], e2[:], corr[:])

            # softmax over free dim, scale C^-0.5 = 0.125 folded into exp
            m = const.tile([64, 1], F32)
            nc.vector.reduce_max(m[:], e2c[:], axis=mybir.AxisListType.X)
            mb = const.tile([64, 1], F32)
            nc.vector.tensor_scalar_mul(mb[:], m[:], -0.125)
            attu = const.tile([64, 64], F32)
            s = const.tile([64, 1], F32)
            nc.scalar.activation(attu[:], e2c[:], Exp, bias=mb[:], scale=0.125,
                                 accum_out=s[:])
            r = const.tile([64, 1], F32)
            nc.vector.reciprocal(r[:], s[:])
            att = const.tile([64, 64], F32)
            nc.vector.tensor_scalar_mul(att[:], attu[:], r[:])

            # att^T (PE transpose), cast fp16, stacked at partitions 0 and 64
            # to match the packed v16 halves
            atp = vps.tile([64, 64], F32, tag="vp")
            nc.tensor.transpose(atp[:], att[:], ident[:])
            att16 = const.tile([128, 64], F16)
            nc.scalar.copy(att16[0:64, :], atp[:])
            nc.scalar.copy(att16[64:128, :], atp[:])

            # out = att @ v over chunk pairs: 3 PSUM slots (2 vp + 1 extra
            # bank), one copy per pair alternating ACT/DVE, DMA per 2048px
            out_sb = None
            for pr in range(NHALF // VC // 2):
                if pr % 3 < 2:
                    op = vps.tile([128, VC], F32, tag="vp")
                else:
                    op = eps.tile([128, VC], F32, tag="op2")
                for h in range(2):
                    psl = slice(h * 64, h * 64 + 64)
                    nc.tensor.matmul(op[psl, :], att16[psl, :],
                                     v16[psl, pr * VC:(pr + 1) * VC],
                                     start=True, stop=True)
                w2 = pr % 2
                if w2 == 0:
                    out_sb = outp.tile([128, PX // 2], F16, tag="out_sb")
                dst = out_sb[:, w2 * VC:(w2 + 1) * VC]
                if pr % 2 == 0:
                    nc.scalar.copy(dst, op[:])
                else:
                    nc.vector.tensor_copy(dst, op[:])
                if w2 == 1:
                    g = pr // 2
                    nc.sync.dma_start(
                        out_d[:, g * (PX // 2):(g + 1) * (PX // 2)], out_sb[:])

    nc.compile()
    return nc


def _get_nc():
    if 'nc' not in _CACHE:
        _CACHE['nc'] = _build_bass()
    return _CACHE['nc']


def kernel(rgb, hsv, lab, Wq, bq, Wk, bk, Wv, bv):
    from concourse.bass_utils import run_bass_kernel_spmd

    nc = _get_nc()

    rgb = np.asarray(rgb, dtype=np.float32)
    hsv = np.asarray(hsv, dtype=np.float32)
    lab = np.asarray(lab, dtype=np.float32)
    Wq = np.asarray(Wq, dtype=np.float32)
    Wk = np.asarray(Wk, dtype=np.float32)
    Wv = np.asarray(Wv, dtype=np.float32)
    bq = np.asarray(bq, dtype=np.float32)
    bk = np.asarray(bk, dtype=np.float32)
    bv = np.asarray(bv, dtype=np.float32)

    # weight prep: [192ch + ones-row, outs] with bias row appended, fp16
    wqk = np.concatenate([Wq.T, Wk.T], axis=1)          # [192, 128]
    bqk = np.concatenate([bq, bk])                      # [128]
    wqk_aug = np.vstack([wqk, bqk[None, :]])            # [193, 128]
    wv_aug = np.vstack([Wv.T, bv[None, :]])             # [193, 64]
    wqk16 = wqk_aug.astype(np.float16)
    wv16 = wv_aug.astype(np.float16)

    # energy correction for the coherent fp16 weight-truncation error:
    # E_true - E_hw ~= N*(dWq^T Wk16 + Wq16^T dWk + dWq^T dWk)
    wq64 = wqk_aug[:, 0:64].astype(np.float64)
    wk64 = wqk_aug[:, 64:128].astype(np.float64)
    wq16_64 = wqk16[:, 0:64].astype(np.float64)
    wk16_64 = wqk16[:, 64:128].astype(np.float64)
    dq = wq64 - wq16_64
    dk = wk64 - wk16_64
    corr = (N * (dq.T @ wk16_64 + wq16_64.T @ dk + dq.T @ dk)).astype(np.float32)

    shared = {
        "w0": np.ascontiguousarray(
            np.concatenate([wqk16[0:128], wv16[0:128]], axis=1)),
        "w1": np.ascontiguousarray(
            np.concatenate([wqk16[128:193], wv16[128:193]], axis=1)),
        "ident": np.eye(64, dtype=np.float32),
        "corr": corr,
    }

    in_maps = []
    for c in range(NCORES):
        b, half = c // 2, c % 2
        hs = slice(half * (H // 2), (half + 1) * (H // 2))
        x01 = np.concatenate([rgb[b, :, hs, :].reshape(C, NHALF),
                              hsv[b, :, hs, :].reshape(C, NHALF)], axis=0)
        in_maps.append({
            "x01": np.ascontiguousarray(x01.astype(np.float16)),
            "x_lab": np.ascontiguousarray(
                lab[b, :, hs, :].reshape(C, NHALF).astype(np.float16)),
            **shared,
        })

    res = run_bass_kernel_spmd(nc, in_maps, core_ids=list(range(NCORES)),
                               **_CACHE.get('run_kwargs', {}))
    _CACHE['last_results'] = res
    _CACHE['last_in_maps'] = in_maps

    out = np.empty((B, C, H, W), dtype=np.float32)
    for c in range(NCORES):
        b, half = c // 2, c % 2
        hs = slice(half * (H // 2), (half + 1) * (H // 2))
        # unpack [128, NHALF/2]: part half*64+ch, col P*512+j -> ch, P*1024+half*512+j
        r = res.results[c]["out"].astype(np.float32).reshape(
            2, 64, NHALF // 1024, 512)
        out[b, :, hs, :] = np.transpose(r, (1, 2, 0, 3)).reshape(C, H // 2, W)
    return out


# revision 16
# speedup vs baseline: 1.4304x; 1.4304x over previous
"""CAM-module (channel attention) Trainium2 kernel — fp16 single-pass version.

Problem: B=4 samples, C=64, H=W=256 (N=65536 px). concat(rgb,hsv,lab) ->
X [192, N] per sample; q/k/v = 1x1-conv projections (W [64,192] + bias);
energy = q @ k^T * C^-0.5 -> softmax over last dim -> out = att @ v.

Sharding: 8 cores = 4 samples x 2 spatial halves (32768 px each). Each
core computes a partial energy over its half; a 16 KiB pairwise AllReduce
([[0,1],[2,3],[4,5],[6,7]]) completes the C x C energy, then each core
computes out for its own half.

Precision: rel-err budget is 2e-2; numpy emulation of this exact scheme
measures ~1.6e-3. X and W are cast to fp16 host-side (RNE, identical to a
device cast); projections are single-pass fp16 matmuls with fp32 PSUM
accumulate. The dominant coherent error — the fixed fp16 weight truncation
dW hitting the Gram matrix G ~= N*I — is corrected host-side:
C = N*(dWq^T Wk16 + Wq16^T dWk + dWq^T dWk) is added to the energy after
the AllReduce. Energy runs as fp16 matmuls (1 cyc/row vs fp32's 4) on the
fp16-cast q/k tiles; residual rounding is incoherent (~0.02 scaled logits),
harmless against typical top-2 logit gaps (~400). The output returns as
fp16 (one more 2^-11 rounding) and is upcast host-side.

Performance structure (TRN2 cost model):
 - fp16 inputs halve the input stream to 12 MiB/core; out fp16 = 2 MiB.
 - rgb+hsv host-packed into one [128, NHALF] tensor -> 1 input DMA/tile.
 - v and out chunks are packed two-per-PSUM-tile at partition offsets 0/64,
   halving PSUM->SBUF copies; out leaves in a [128, NHALF/2] fp16 layout
   the host unpacks. PSUM->SBUF copies alternate ACT/DVE.
 - qk PSUM groups span 2 banks ([128,1024]) -> one fp16 cast per 8 subtiles.
   The cast scatters the group into [q0..q7 | k0..k7] regions so the energy
   runs as 4 paired matmuls per group (lhsT=[q_2s|q_2s+1], rhs=[k_2s|k_2s+1]
   into a [128,128] accumulator whose diagonal blocks hold the energy) —
   half the energy instruction count of the unpaired form.
 - PE/tile(2048px): proj 2x128x16 + energy 4x128 + v 4x512 = 9216 cyc
   (~3.8us warm) -> the main loop is PE-bound over a ~2.2us/tile DMA floor.
 - v is deferred 8 tiles so ~13us of v matmuls overlap the AllReduce.
 - ~2000 instructions total vs ~2500 in the fp16-hi/lo 3-pass predecessor;
   TimelineSim: 96.6us/core vs 218us for the predecessor.
"""

import sys
import numpy as np

if '/opt/trn_rl_repo' not in sys.path:
    sys.path.insert(0, '/opt/trn_rl_repo')

B, C, H, W = 4, 64, 256, 256
N = H * W                 # 65536 px per sample
NHALF = N // 2            # 32768 px per core
PX = 2048                 # streaming tile (px)
NIT = NHALF // PX         # 16
SUB = 128                 # proj subtile (px) = matmul M
NSUB = PX // SUB          # 16
VC = 512                  # v / out chunk (px) = matmul N
NVC = PX // VC            # 4
NCORES = 8
DEFER = 8                 # v-tiles deferred into the AllReduce window

_CACHE = {}


def _build_bass(single_core=False):
    import concourse.bacc as bacc
    import concourse.mybir as mybir
    from concourse import tile

    F32 = mybir.dt.float32
    F16 = mybir.dt.float16
    Exp = mybir.ActivationFunctionType.Exp

    nc = bacc.Bacc("TRN2", target_bir_lowering=False, debug=False,
                   enable_asserts=False,
                   num_devices=1 if single_core else NCORES)

    x01_d = nc.dram_tensor("x01", [128, NHALF], F16, kind="ExternalInput").ap()
    xb_d = nc.dram_tensor("x_lab", [64, NHALF], F16, kind="ExternalInput").ap()
    # packed fp16 weights: cols [wqk 0:128 | wv 128:192]
    w0_d = nc.dram_tensor("w0", [128, 192], F16, kind="ExternalInput").ap()
    w1_d = nc.dram_tensor("w1", [65, 192], F16, kind="ExternalInput").ap()
    ident_d = nc.dram_tensor("ident", [64, 64], F32, kind="ExternalInput").ap()
    corr_d = nc.dram_tensor("corr", [64, 64], F32, kind="ExternalInput").ap()
    # out, partition-pair packed: part p<64 = ch p, even 512-px chunk of the
    # 1024-col block; part p>=64 = ch p-64, odd chunk (host unpacks)
    out_d = nc.dram_tensor("out", [128, NHALF // 2], F16,
                           kind="ExternalOutput").ap()

    with tile.TileContext(nc) as tc:
        with tc.tile_pool(name="const", bufs=1) as const, \
             tc.tile_pool(name="s16", bufs=DEFER + 1) as s16, \
             tc.tile_pool(name="qk", bufs=4) as qkpool, \
             tc.tile_pool(name="outp", bufs=4) as outp, \
             tc.tile_pool(name="qkps", bufs=2, space="PSUM") as qkps, \
             tc.tile_pool(name="vps", bufs=2, space="PSUM") as vps, \
             tc.tile_pool(name="eps", bufs=1, space="PSUM") as eps, \
             tc.tile_pool(name="dram", bufs=1, space="DRAM") as dram:

            w0 = const.tile([128, 192], F16)
            w1 = const.tile([65, 192], F16)
            ident = const.tile([64, 64], F32)
            corr = const.tile([64, 64], F32)
            nc.scalar.dma_start(w0[:], w0_d[:])
            nc.scalar.dma_start(w1[:], w1_d[:])
            nc.scalar.dma_start(ident[:], ident_d[:])
            nc.scalar.dma_start(corr[:], corr_d[:])
            wqk0, wv0 = w0[:, 0:128], w0[:, 128:192]
            wqk1, wv1 = w1[:, 0:128], w1[:, 128:192]

            # preload the ACT Exp table set off the critical path (~2.7us)
            warm = const.tile([1, 1], F32)
            nc.gpsimd.memset(warm[:], 0.0)
            nc.scalar.activation(warm[:], warm[:], Exp)

            # v, partition-pair packed like out_d (4 MiB fp16)
            v16 = const.tile([128, NHALF // 2], F16)
            # paired energy accumulator: subtile pairs (2s, 2s+1) matmul as
            # lhsT=[q_2s|q_2s+1], rhs=[k_2s|k_2s+1]; the diagonal 64x64
            # blocks accumulate the true energy, off-diagonal is discarded
            ep = eps.tile([128, 128], F32)

            def v_block(vit, vx0h, vx1h):
                for pr in range(NVC // 2):        # chunk pairs
                    vp = vps.tile([128, VC], F32, tag="vp")
                    for h in range(2):
                        vsl = slice((2 * pr + h) * VC, (2 * pr + h + 1) * VC)
                        psl = slice(h * 64, h * 64 + 64)
                        nc.tensor.matmul(vp[psl, :], wv0[:], vx0h[:, vsl],
                                         start=True, stop=False)
                        nc.tensor.matmul(vp[psl, :], wv1[:], vx1h[:, vsl],
                                         start=False, stop=True)
                    blk = vit * (PX // 2) + pr * VC
                    if pr % 2 == 0:
                        nc.scalar.copy(v16[:, blk:blk + VC], vp[:])
                    else:
                        nc.vector.tensor_copy(v16[:, blk:blk + VC], vp[:])

            pending = []
            for it in range(NIT):
                sl = slice(it * PX, (it + 1) * PX)
                x0h = s16.tile([128, PX], F16, tag="x0h")
                nc.sync.dma_start(x0h[:], x01_d[:, sl])
                x1h = s16.tile([65, PX], F16, tag="x1h")
                nc.sync.dma_start(x1h[0:64, :], xb_d[:, sl])
                if it < DEFER + 1:
                    # ones rows live in the round-robin pool slots; later
                    # iterations reuse them untouched
                    nc.gpsimd.memset(x1h[64:65, :], 1.0)

                for grp in range(NSUB // 8):   # 8 proj subtiles per PSUM pair
                    qkp = qkps.tile([128, 1024], F32, tag="qkp")
                    for s8 in range(8):
                        sb = grp * 8 + s8
                        ssl = slice(sb * SUB, (sb + 1) * SUB)
                        osl = slice(s8 * 128, (s8 + 1) * 128)
                        nc.tensor.matmul(qkp[:, osl], x0h[:, ssl], wqk0[:],
                                         start=True, stop=False)
                        nc.tensor.matmul(qkp[:, osl], x1h[:, ssl], wqk1[:],
                                         start=False, stop=True)
                    # scatter-cast: qkp [s][q|k] -> qk_sb [q0..q7 | k0..k7]
                    # so energy subtile-pair operands are contiguous
                    qk_sb = qkpool.tile([128, 1024], F16, tag="qk_sb")
                    dst = qk_sb[:].rearrange("p (qk s b) -> p s qk b",
                                             qk=2, s=8, b=64)
                    srcv = qkp[:].rearrange("p (s qk b) -> p s qk b",
                                            s=8, qk=2, b=64)
                    if grp == 0:
                        nc.scalar.copy(dst, srcv)
                    else:
                        nc.vector.tensor_copy(dst, srcv)
                    for s4 in range(4):
                        first = (it == 0 and grp == 0 and s4 == 0)
                        last = (it == NIT - 1 and grp == NSUB // 8 - 1 and s4 == 3)
                        nc.tensor.matmul(ep[:], qk_sb[:, s4 * 128:s4 * 128 + 128],
                                         qk_sb[:, 512 + s4 * 128:512 + s4 * 128 + 128],
                                         start=first, stop=last)

                if it >= DEFER:
                    v_block(it - DEFER, *pending.pop(0))
                pending.append((x0h, x1h))

            # partial energy -> pairwise AllReduce; the deferred v matmuls
            # below keep PE busy while the collective is in flight
            ep_sb = const.tile([128, 128], F32)
            nc.scalar.copy(ep_sb[:], ep[:])
            ebr = const.tile([64, 64], F32)
            nc.sync.dma_start(ebr[:], ep_sb[64:128, 64:128])
            e_sb = const.tile([64, 64], F32)
            nc.vector.tensor_add(e_sb[:], ep_sb[0:64, 0:64], ebr[:])
            bi = dram.tile([64, 64], F32)
            bo = dram.tile([64, 64], F32)
            nc.sync.dma_start(bi[:], e_sb[:])
            if single_core:
                nc.gpsimd.dma_start(bo[:], bi[:])
            else:
                nc.gpsimd.collective_compute(
                    "AllReduce", mybir.AluOpType.add,
                    replica_groups=[[0, 1], [2, 3], [4, 5], [6, 7]],
                    ins=[bi.opt()], outs=[bo.opt()],
                )

            for d in range(DEFER):
                v_block(NIT - DEFER + d, *pending[d])

            e2 = const.tile([64, 64], F32)
            nc.sync.dma_start(e2[:], bo[:])
            # fp16-weight-truncation correction (see module docstring)
            e2c = const.tile([64, 64], F32)
            nc.vector.tensor_add(e2c[:], e2[:], corr[:])

            # softmax over free dim, scale C^-0.5 = 0.125 folded into exp
            m = const.tile([64, 1], F32)
            nc.vector.reduce_max(m[:], e2c[:], axis=mybir.AxisListType.X)
            mb = const.tile([64, 1], F32)
            nc.vector.tensor_scalar_mul(mb[:], m[:], -0.125)
            attu = const.tile([64, 64], F32)
            s = const.tile([64, 1], F32)
            nc.scalar.activation(attu[:], e2c[:], Exp, bias=mb[:], scale=0.125,
                                 accum_out=s[:])
            r = const.tile([64, 1], F32)
            nc.vector.reciprocal(r[:], s[:])
            att = const.tile([64, 64], F32)
            nc.vector.tensor_scalar_mul(att[:], attu[:], r[:])

            # att^T (PE transpose), cast fp16, stacked at partitions 0 and 64
            # to match the packed v16 halves
            atp = vps.tile([64, 64], F32, tag="vp")
            nc.tensor.transpose(atp[:], att[:], ident[:])
            att16 = const.tile([128, 64], F16)
            nc.scalar.copy(att16[0:64, :], atp[:])
            nc.scalar.copy(att16[64:128, :], atp[:])

            # out = att @ v over chunk pairs: 3 PSUM slots (2 vp + 1 extra
            # bank), one copy per pair alternating ACT/DVE, DMA per 2048px
            out_sb = None
            for pr in range(NHALF // VC // 2):
                if pr % 3 < 2:
                    op = vps.tile([128, VC], F32, tag="vp")
                else:
                    op = eps.tile([128, VC], F32, tag="op2")
                for h in range(2):
                    psl = slice(h * 64, h * 64 + 64)
                    nc.tensor.matmul(op[psl, :], att16[psl, :],
                                     v16[psl, pr * VC:(pr + 1) * VC],
                                     start=True, stop=True)
                w2 = pr % 2
                if w2 == 0:
                    out_sb = outp.tile([128, PX // 2], F16, tag="out_sb")
                dst = out_sb[:, w2 * VC:(w2 + 1) * VC]
                if pr % 2 == 0:
                    nc.scalar.copy(dst, op[:])
                else:
                    nc.vector.tensor_copy(dst, op[:])
                if w2 == 1:
                    g = pr // 2
                    nc.sync.dma_start(
                        out_d[:, g * (PX // 2):(g + 1) * (PX // 2)], out_sb[:])

    nc.compile()
    return nc


def _get_nc():
    if 'nc' not in _CACHE:
        _CACHE['nc'] = _build_bass()
    return _CACHE['nc']


def kernel(rgb, hsv, lab, Wq, bq, Wk, bk, Wv, bv):
    from concourse.bass_utils import run_bass_kernel_spmd

    nc = _get_nc()

    rgb = np.asarray(rgb, dtype=np.float32)
    hsv = np.asarray(hsv, dtype=np.float32)
    lab = np.asarray(lab, dtype=np.float32)
    Wq = np.asarray(Wq, dtype=np.float32)
    Wk = np.asarray(Wk, dtype=np.float32)
    Wv = np.asarray(Wv, dtype=np.float32)
    bq = np.asarray(bq, dtype=np.float32)
    bk = np.asarray(bk, dtype=np.float32)
    bv = np.asarray(bv, dtype=np.float32)

    # weight prep: [192ch + ones-row, outs] with bias row appended, fp16
    wqk = np.concatenate([Wq.T, Wk.T], axis=1)          # [192, 128]
    bqk = np.concatenate([bq, bk])                      # [128]
    wqk_aug = np.vstack([wqk, bqk[None, :]])            # [193, 128]
    wv_aug = np.vstack([Wv.T, bv[None, :]])             # [193, 64]
    wqk16 = wqk_aug.astype(np.float16)
    wv16 = wv_aug.astype(np.float16)

    # energy correction for the coherent fp16 weight-truncation error:
    # E_true - E_hw ~= N*(dWq^T Wk16 + Wq16^T dWk + dWq^T dWk)
    wq64 = wqk_aug[:, 0:64].astype(np.float64)
    wk64 = wqk_aug[:, 64:128].astype(np.float64)
    wq16_64 = wqk16[:, 0:64].astype(np.float64)
    wk16_64 = wqk16[:, 64:128].astype(np.float64)
    dq = wq64 - wq16_64
    dk = wk64 - wk16_64
    corr = (N * (dq.T @ wk16_64 + wq16_64.T @ dk + dq.T @ dk)).astype(np.float32)

    shared = {
        "w0": np.ascontiguousarray(
            np.concatenate([wqk16[0:128], wv16[0:128]], axis=1)),
        "w1": np.ascontiguousarray(
            np.concatenate([wqk16[128:193], wv16[128:193]], axis=1)),
        "ident": np.eye(64, dtype=np.float32),
        "corr": corr,
    }

    in_maps = []
    for c in range(NCORES):
        b, half = c // 2, c % 2
        hs = slice(half * (H // 2), (half + 1) * (H // 2))
        x01 = np.concatenate([rgb[b, :, hs, :].reshape(C, NHALF),
                              hsv[b, :, hs, :].reshape(C, NHALF)], axis=0)
        in_maps.append({
            "x01": np.ascontiguousarray(x01.astype(np.float16)),
            "x_lab": np.ascontiguousarray(
                lab[b, :, hs, :].reshape(C, NHALF).astype(np.float16)),
            **shared,
        })

    res = run_bass_kernel_spmd(nc, in_maps, core_ids=list(range(NCORES)),
                               **_CACHE.get('run_kwargs', {}))
    _CACHE['last_results'] = res
    _CACHE['last_in_maps'] = in_maps

    out = np.empty((B, C, H, W), dtype=np.float32)
    for c in range(NCORES):
        b, half = c // 2, c % 2
        hs = slice(half * (H // 2), (half + 1) * (H // 2))
        # unpack [128, NHALF/2]: part half*64+ch, col P*512+j -> ch, P*1024+half*512+j
        r = res.results[c]["out"].astype(np.float32).reshape(
            2, 64, NHALF // 1024, 512)
        out[b, :, hs, :] = np.transpose(r, (1, 2, 0, 3)).reshape(C, H // 2, W)
    return out


# revision 23
# speedup vs baseline: 1.6295x; 1.1391x over previous
"""CAM-module (channel attention) Trainium2 kernel — fp16 single-pass version.

Problem: B=4 samples, C=64, H=W=256 (N=65536 px). concat(rgb,hsv,lab) ->
X [192, N] per sample; q/k/v = 1x1-conv projections (W [64,192] + bias);
energy = q @ k^T * C^-0.5 -> softmax over last dim -> out = att @ v.

Sharding: 8 cores = 4 samples x 2 spatial halves (32768 px each). Each
core computes a partial energy over its half; a 16 KiB pairwise AllReduce
([[0,1],[2,3],[4,5],[6,7]]) completes the C x C energy, then each core
computes out for its own half.

Precision: rel-err budget is 2e-2; numpy emulation of this exact scheme
measures ~1.6e-3. X and W are cast to fp16 host-side (RNE, identical to a
device cast); projections are single-pass fp16 matmuls with fp32 PSUM
accumulate. The dominant coherent error — the fixed fp16 weight truncation
dW hitting the Gram matrix G ~= N*I — is corrected host-side:
C = N*(dWq^T Wk16 + Wq16^T dWk + dWq^T dWk) is added to the energy after
the AllReduce. Energy runs as fp16 matmuls (1 cyc/row vs fp32's 4) on the
fp16-cast q/k tiles; residual rounding is incoherent (~0.02 scaled logits),
harmless against typical top-2 logit gaps (~400). The output returns as
fp16 (one more 2^-11 rounding) and is upcast host-side.

Performance structure (TRN2 cost model):
 - fp16 inputs halve the input stream to 12 MiB/core; out fp16 = 2 MiB.
 - rgb+hsv host-packed into one [128, NHALF] tensor -> 1 input DMA/tile.
 - v and out chunks are packed two-per-PSUM-tile at partition offsets 0/64,
   halving PSUM->SBUF copies; out leaves in a [128, NHALF/2] fp16 layout
   the host unpacks. PSUM->SBUF copies alternate ACT/DVE.
 - qk PSUM groups span 2 banks ([128,1024]) -> one fp16 cast per 8 subtiles.
   The cast scatters the group into [q0..q7 | k0..k7] regions so the energy
   runs as 4 paired matmuls per group (lhsT=[q_2s|q_2s+1], rhs=[k_2s|k_2s+1]
   into a [128,128] accumulator whose diagonal blocks hold the energy) —
   half the energy instruction count of the unpaired form.
 - PE/tile(2048px): proj 2x128x16 + energy 4x128 + v 4x512 = 9216 cyc
   (~3.8us warm) -> the main loop is PE-bound over a ~2.2us/tile DMA floor.
 - v is deferred 8 tiles so ~13us of v matmuls overlap the AllReduce.
 - ~2000 instructions total vs ~2500 in the fp16-hi/lo 3-pass predecessor;
   TimelineSim: 96.6us/core vs 218us for the predecessor.
"""

import sys
import numpy as np

if '/opt/trn_rl_repo' not in sys.path:
    sys.path.insert(0, '/opt/trn_rl_repo')

B, C, H, W = 4, 64, 256, 256
N = H * W                 # 65536 px per sample
NHALF = N // 2            # 32768 px per core
PX = 2048                 # streaming tile (px)
NIT = NHALF // PX         # 16
SUB = 128                 # proj subtile (px) = matmul M
NSUB = PX // SUB          # 16
VC = 512                  # v / out chunk (px) = matmul N
NVC = PX // VC            # 4
NCORES = 8
DEFER = 8                 # v-tiles deferred into the AllReduce window

_CACHE = {}


def _build_bass(single_core=False):
    import concourse.bacc as bacc
    import concourse.mybir as mybir
    from concourse import tile

    F32 = mybir.dt.float32
    F16 = mybir.dt.float16
    Exp = mybir.ActivationFunctionType.Exp

    nc = bacc.Bacc("TRN2", target_bir_lowering=False, debug=False,
                   enable_asserts=False,
                   num_devices=1 if single_core else NCORES)

    x01_d = nc.dram_tensor("x01", [128, NHALF], F16, kind="ExternalInput").ap()
    xb_d = nc.dram_tensor("x_lab", [64, NHALF], F16, kind="ExternalInput").ap()
    # packed fp16 weights: cols [wqk 0:128 | wv 128:192]
    w0_d = nc.dram_tensor("w0", [128, 192], F16, kind="ExternalInput").ap()
    w1_d = nc.dram_tensor("w1", [65, 192], F16, kind="ExternalInput").ap()
    ident_d = nc.dram_tensor("ident", [64, 64], F32, kind="ExternalInput").ap()
    corr_d = nc.dram_tensor("corr", [64, 64], F32, kind="ExternalInput").ap()
    # out, partition-pair packed: part p<64 = ch p, even 512-px chunk of the
    # 1024-col block; part p>=64 = ch p-64, odd chunk (host unpacks)
    out_d = nc.dram_tensor("out", [128, NHALF // 2], F16,
                           kind="ExternalOutput").ap()

    with tile.TileContext(nc) as tc:
        with tc.tile_pool(name="const", bufs=1) as const, \
             tc.tile_pool(name="s16", bufs=DEFER + 1) as s16, \
             tc.tile_pool(name="qk", bufs=8) as qkpool, \
             tc.tile_pool(name="outp", bufs=8) as outp, \
             tc.tile_pool(name="qkps", bufs=2, space="PSUM") as qkps, \
             tc.tile_pool(name="vps", bufs=2, space="PSUM") as vps, \
             tc.tile_pool(name="eps", bufs=1, space="PSUM") as eps, \
             tc.tile_pool(name="dram", bufs=1, space="DRAM") as dram:

            w0 = const.tile([128, 192], F16)
            w1 = const.tile([65, 192], F16)
            ident = const.tile([64, 64], F32)
            corr = const.tile([64, 64], F32)
            nc.scalar.dma_start(w0[:], w0_d[:])
            nc.scalar.dma_start(w1[:], w1_d[:])
            nc.scalar.dma_start(ident[:], ident_d[:])
            nc.scalar.dma_start(corr[:], corr_d[:])
            wqk0, wv0 = w0[:, 0:128], w0[:, 128:192]
            wqk1, wv1 = w1[:, 0:128], w1[:, 128:192]

            # preload the ACT Exp table set off the critical path (~2.7us)
            warm = const.tile([1, 1], F32)
            nc.gpsimd.memset(warm[:], 0.0)
            nc.scalar.activation(warm[:], warm[:], Exp)

            # v, partition-pair packed like out_d (4 MiB fp16)
            v16 = const.tile([128, NHALF // 2], F16)
            # paired energy accumulator: subtile pairs (2s, 2s+1) matmul as
            # lhsT=[q_2s|q_2s+1], rhs=[k_2s|k_2s+1]; the diagonal 64x64
            # blocks accumulate the true energy, off-diagonal is discarded
            ep = eps.tile([128, 128], F32)

            def v_block(vit, vx0h, vx1h):
                for pr in range(NVC // 2):        # chunk pairs
                    vp = vps.tile([128, VC], F32, tag="vp")
                    for h in range(2):
                        vsl = slice((2 * pr + h) * VC, (2 * pr + h + 1) * VC)
                        psl = slice(h * 64, h * 64 + 64)
                        nc.tensor.matmul(vp[psl, :], wv0[:], vx0h[:, vsl],
                                         start=True, stop=False)
                        nc.tensor.matmul(vp[psl, :], wv1[:], vx1h[:, vsl],
                                         start=False, stop=True)
                    blk = vit * (PX // 2) + pr * VC
                    if pr % 2 == 0:
                        nc.scalar.copy(v16[:, blk:blk + VC], vp[:])
                    else:
                        nc.vector.tensor_copy(v16[:, blk:blk + VC], vp[:])

            pending = []
            for it in range(NIT):
                sl = slice(it * PX, (it + 1) * PX)
                x0h = s16.tile([128, PX], F16, tag="x0h")
                nc.sync.dma_start(x0h[:], x01_d[:, sl])
                x1h = s16.tile([65, PX], F16, tag="x1h")
                nc.sync.dma_start(x1h[0:64, :], xb_d[:, sl])
                if it < DEFER + 1:
                    # ones rows live in the round-robin pool slots; later
                    # iterations reuse them untouched
                    nc.gpsimd.memset(x1h[64:65, :], 1.0)

                for grp in range(NSUB // 8):   # 8 proj subtiles per PSUM pair
                    qkp = qkps.tile([128, 1024], F32, tag="qkp")
                    for s8 in range(8):
                        sb = grp * 8 + s8
                        ssl = slice(sb * SUB, (sb + 1) * SUB)
                        osl = slice(s8 * 128, (s8 + 1) * 128)
                        nc.tensor.matmul(qkp[:, osl], x0h[:, ssl], wqk0[:],
                                         start=True, stop=False)
                        nc.tensor.matmul(qkp[:, osl], x1h[:, ssl], wqk1[:],
                                         start=False, stop=True)
                    # scatter-cast: qkp [s][q|k] -> qk_sb [q0..q7 | k0..k7]
                    # so energy subtile-pair operands are contiguous
                    qk_sb = qkpool.tile([128, 1024], F16, tag="qk_sb")
                    dst = qk_sb[:].rearrange("p (qk s b) -> p s qk b",
                                             qk=2, s=8, b=64)
                    srcv = qkp[:].rearrange("p (s qk b) -> p s qk b",
                                            s=8, qk=2, b=64)
                    if grp == 0:
                        nc.scalar.copy(dst, srcv)
                    else:
                        nc.vector.tensor_copy(dst, srcv)
                    for s4 in range(4):
                        first = (it == 0 and grp == 0 and s4 == 0)
                        last = (it == NIT - 1 and grp == NSUB // 8 - 1 and s4 == 3)
                        nc.tensor.matmul(ep[:], qk_sb[:, s4 * 128:s4 * 128 + 128],
                                         qk_sb[:, 512 + s4 * 128:512 + s4 * 128 + 128],
                                         start=first, stop=last)

                if it >= DEFER:
                    v_block(it - DEFER, *pending.pop(0))
                pending.append((x0h, x1h))

            # partial energy -> pairwise AllReduce; the deferred v matmuls
            # below keep PE busy while the collective is in flight
            ep_sb = const.tile([128, 128], F32)
            nc.scalar.copy(ep_sb[:], ep[:])
            ebr = const.tile([64, 64], F32)
            nc.sync.dma_start(ebr[:], ep_sb[64:128, 64:128])
            e_sb = const.tile([64, 64], F32)
            nc.vector.tensor_add(e_sb[:], ep_sb[0:64, 0:64], ebr[:])
            bi = dram.tile([64, 64], F32)
            bo = dram.tile([64, 64], F32)
            nc.sync.dma_start(bi[:], e_sb[:])
            if single_core:
                nc.gpsimd.dma_start(bo[:], bi[:])
            else:
                nc.gpsimd.collective_compute(
                    "AllReduce", mybir.AluOpType.add,
                    replica_groups=[[0, 1], [2, 3], [4, 5], [6, 7]],
                    ins=[bi.opt()], outs=[bo.opt()],
                )

            for d in range(DEFER):
                v_block(NIT - DEFER + d, *pending[d])

            e2 = const.tile([64, 64], F32)
            nc.sync.dma_start(e2[:], bo[:])
            # fp16-weight-truncation correction (see module docstring)
            e2c = const.tile([64, 64], F32)
            nc.vector.tensor_add(e2c[:], e2[:], corr[:])

            # softmax over free dim, scale C^-0.5 = 0.125 folded into exp
            m = const.tile([64, 1], F32)
            nc.vector.reduce_max(m[:], e2c[:], axis=mybir.AxisListType.X)
            mb = const.tile([64, 1], F32)
            nc.vector.tensor_scalar_mul(mb[:], m[:], -0.125)
            attu = const.tile([64, 64], F32)
            s = const.tile([64, 1], F32)
            nc.scalar.activation(attu[:], e2c[:], Exp, bias=mb[:], scale=0.125,
                                 accum_out=s[:])
            r = const.tile([64, 1], F32)
            nc.vector.reciprocal(r[:], s[:])
            att = const.tile([64, 64], F32)
            nc.vector.tensor_scalar_mul(att[:], attu[:], r[:])

            # att^T (PE transpose), cast fp16, stacked at partitions 0 and 64
            # to match the packed v16 halves
            atp = vps.tile([64, 64], F32, tag="vp")
            nc.tensor.transpose(atp[:], att[:], ident[:])
            att16 = const.tile([128, 64], F16)
            nc.scalar.copy(att16[0:64, :], atp[:])
            nc.scalar.copy(att16[64:128, :], atp[:])

            # out = att @ v over chunk pairs: 3 PSUM slots (2 vp + 1 extra
            # bank), one copy per pair alternating ACT/DVE, DMA per 2048px
            out_sb = None
            for pr in range(NHALF // VC // 2):
                if pr % 3 < 2:
                    op = vps.tile([128, VC], F32, tag="vp")
                else:
                    op = eps.tile([128, VC], F32, tag="op2")
                for h in range(2):
                    psl = slice(h * 64, h * 64 + 64)
                    nc.tensor.matmul(op[psl, :], att16[psl, :],
                                     v16[psl, pr * VC:(pr + 1) * VC],
                                     start=True, stop=True)
                w2 = pr % 2
                if w2 == 0:
                    out_sb = outp.tile([128, PX // 2], F16, tag="out_sb")
                dst = out_sb[:, w2 * VC:(w2 + 1) * VC]
                if pr % 2 == 0:
                    nc.scalar.copy(dst, op[:])
                else:
                    nc.vector.tensor_copy(dst, op[:])
                if w2 == 1:
                    g = pr // 2
                    nc.sync.dma_start(
                        out_d[:, g * (PX // 2):(g + 1) * (PX // 2)], out_sb[:])

    nc.compile()
    return nc


def _get_nc():
    if 'nc' not in _CACHE:
        _CACHE['nc'] = _build_bass()
    return _CACHE['nc']


def kernel(rgb, hsv, lab, Wq, bq, Wk, bk, Wv, bv):
    from concourse.bass_utils import run_bass_kernel_spmd

    nc = _get_nc()

    rgb = np.asarray(rgb, dtype=np.float32)
    hsv = np.asarray(hsv, dtype=np.float32)
    lab = np.asarray(lab, dtype=np.float32)
    Wq = np.asarray(Wq, dtype=np.float32)
    Wk = np.asarray(Wk, dtype=np.float32)
    Wv = np.asarray(Wv, dtype=np.float32)
    bq = np.asarray(bq, dtype=np.float32)
    bk = np.asarray(bk, dtype=np.float32)
    bv = np.asarray(bv, dtype=np.float32)

    # weight prep: [192ch + ones-row, outs] with bias row appended, fp16
    wqk = np.concatenate([Wq.T, Wk.T], axis=1)          # [192, 128]
    bqk = np.concatenate([bq, bk])                      # [128]
    wqk_aug = np.vstack([wqk, bqk[None, :]])            # [193, 128]
    wv_aug = np.vstack([Wv.T, bv[None, :]])             # [193, 64]
    wqk16 = wqk_aug.astype(np.float16)
    wv16 = wv_aug.astype(np.float16)

    # energy correction for the coherent fp16 weight-truncation error:
    # E_true - E_hw ~= N*(dWq^T Wk16 + Wq16^T dWk + dWq^T dWk)
    wq64 = wqk_aug[:, 0:64].astype(np.float64)
    wk64 = wqk_aug[:, 64:128].astype(np.float64)
    wq16_64 = wqk16[:, 0:64].astype(np.float64)
    wk16_64 = wqk16[:, 64:128].astype(np.float64)
    dq = wq64 - wq16_64
    dk = wk64 - wk16_64
    corr = (N * (dq.T @ wk16_64 + wq16_64.T @ dk + dq.T @ dk)).astype(np.float32)

    shared = {
        "w0": np.ascontiguousarray(
            np.concatenate([wqk16[0:128], wv16[0:128]], axis=1)),
        "w1": np.ascontiguousarray(
            np.concatenate([wqk16[128:193], wv16[128:193]], axis=1)),
        "ident": np.eye(64, dtype=np.float32),
        "corr": corr,
    }

    in_maps = []
    for c in range(NCORES):
        b, half = c // 2, c % 2
        hs = slice(half * (H // 2), (half + 1) * (H // 2))
        x01 = np.concatenate([rgb[b, :, hs, :].reshape(C, NHALF),
                              hsv[b, :, hs, :].reshape(C, NHALF)], axis=0)
        in_maps.append({
            "x01": np.ascontiguousarray(x01.astype(np.float16)),
            "x_lab": np.ascontiguousarray(
                lab[b, :, hs, :].reshape(C, NHALF).astype(np.float16)),
            **shared,
        })

    res = run_bass_kernel_spmd(nc, in_maps, core_ids=list(range(NCORES)),
                               **_CACHE.get('run_kwargs', {}))
    _CACHE['last_results'] = res
    _CACHE['last_in_maps'] = in_maps

    out = np.empty((B, C, H, W), dtype=np.float32)
    for c in range(NCORES):
        b, half = c // 2, c % 2
        hs = slice(half * (H // 2), (half + 1) * (H // 2))
        # unpack [128, NHALF/2]: part half*64+ch, col P*512+j -> ch, P*1024+half*512+j
        r = res.results[c]["out"].astype(np.float32).reshape(
            2, 64, NHALF // 1024, 512)
        out[b, :, hs, :] = np.transpose(r, (1, 2, 0, 3)).reshape(C, H // 2, W)
    return out


# revision 24
# speedup vs baseline: 3.5519x; 2.1798x over previous
"""CAM-module (channel attention) Trainium2 kernel — fp16 single-pass version.

Problem: B=4 samples, C=64, H=W=256 (N=65536 px). concat(rgb,hsv,lab) ->
X [192, N] per sample; q/k/v = 1x1-conv projections (W [64,192] + bias);
energy = q @ k^T * C^-0.5 -> softmax over last dim -> out = att @ v.

Sharding: 8 cores = 4 samples x 2 spatial halves (32768 px each). Each
core computes a partial energy over its half; a 16 KiB pairwise AllReduce
([[0,1],[2,3],[4,5],[6,7]]) completes the C x C energy, then each core
computes out for its own half.

Precision: rel-err budget is 2e-2; numpy emulation of this exact scheme
measures ~1.6e-3. X and W are cast to fp16 host-side (RNE, identical to a
device cast); projections are single-pass fp16 matmuls with fp32 PSUM
accumulate. The dominant coherent error — the fixed fp16 weight truncation
dW hitting the Gram matrix G ~= N*I — is corrected host-side:
C = N*(dWq^T Wk16 + Wq16^T dWk + dWq^T dWk) is added to the energy after
the AllReduce. Energy runs as fp16 matmuls (1 cyc/row vs fp32's 4) on the
fp16-cast q/k tiles; residual rounding is incoherent (~0.02 scaled logits),
harmless against typical top-2 logit gaps (~400). The output returns as
fp16 (one more 2^-11 rounding) and is upcast host-side.

Performance structure (TRN2 cost model):
 - fp16 inputs halve the input stream to 12 MiB/core; out fp16 = 2 MiB.
 - rgb+hsv host-packed into one [128, NHALF] tensor -> 1 input DMA/tile.
 - v and out chunks are packed two-per-PSUM-tile at partition offsets 0/64,
   halving PSUM->SBUF copies; out leaves in a [128, NHALF/2] fp16 layout
   the host unpacks. PSUM->SBUF copies alternate ACT/DVE.
 - qk PSUM groups span 2 banks ([128,1024]) -> one fp16 cast per 8 subtiles.
   The cast scatters the group into [q0..q7 | k0..k7] regions so the energy
   runs as 4 paired matmuls per group (lhsT=[q_2s|q_2s+1], rhs=[k_2s|k_2s+1]
   into a [128,128] accumulator whose diagonal blocks hold the energy) —
   half the energy instruction count of the unpaired form.
 - PE/tile(2048px): proj 2x128x16 + energy 4x128 + v 4x512 = 9216 cyc
   (~3.8us warm) -> the main loop is PE-bound over a ~2.2us/tile DMA floor.
 - v is deferred 8 tiles so ~13us of v matmuls overlap the AllReduce.
 - ~2000 instructions total vs ~2500 in the fp16-hi/lo 3-pass predecessor;
   TimelineSim: 94.3us/core vs 218us for the predecessor.
"""

import sys
import numpy as np

if '/opt/trn_rl_repo' not in sys.path:
    sys.path.insert(0, '/opt/trn_rl_repo')

B, C, H, W = 4, 64, 256, 256
N = H * W                 # 65536 px per sample
NHALF = N // 2            # 32768 px per core
PX = 2048                 # streaming tile (px)
NIT = NHALF // PX         # 16
SUB = 128                 # proj subtile (px) = matmul M
NSUB = PX // SUB          # 16
VC = 512                  # v / out chunk (px) = matmul N
NVC = PX // VC            # 4
NCORES = 8
DEFER = 8                 # v-tiles deferred into the AllReduce window

_CACHE = {}


def _build_bass(single_core=False):
    import concourse.bacc as bacc
    import concourse.mybir as mybir
    from concourse import tile

    F32 = mybir.dt.float32
    F16 = mybir.dt.float16
    Exp = mybir.ActivationFunctionType.Exp

    nc = bacc.Bacc("TRN2", target_bir_lowering=False, debug=False,
                   enable_asserts=False,
                   num_devices=1 if single_core else NCORES)

    x01_d = nc.dram_tensor("x01", [128, NHALF], F16, kind="ExternalInput").ap()
    xb_d = nc.dram_tensor("x_lab", [64, NHALF], F16, kind="ExternalInput").ap()
    # packed fp16 weights: cols [wqk 0:128 | wv 128:192]
    w0_d = nc.dram_tensor("w0", [128, 192], F16, kind="ExternalInput").ap()
    w1_d = nc.dram_tensor("w1", [65, 192], F16, kind="ExternalInput").ap()
    ident_d = nc.dram_tensor("ident", [64, 64], F32, kind="ExternalInput").ap()
    corr_d = nc.dram_tensor("corr", [64, 64], F32, kind="ExternalInput").ap()
    # out, partition-pair packed: part p<64 = ch p, even 512-px chunk of the
    # 1024-col block; part p>=64 = ch p-64, odd chunk (host unpacks)
    out_d = nc.dram_tensor("out", [128, NHALF // 2], F16,
                           kind="ExternalOutput").ap()

    with tile.TileContext(nc) as tc:
        with tc.tile_pool(name="const", bufs=1) as const, \
             tc.tile_pool(name="s16", bufs=DEFER + 1) as s16, \
             tc.tile_pool(name="qk", bufs=8) as qkpool, \
             tc.tile_pool(name="outp", bufs=8) as outp, \
             tc.tile_pool(name="qkps", bufs=2, space="PSUM") as qkps, \
             tc.tile_pool(name="vps", bufs=2, space="PSUM") as vps, \
             tc.tile_pool(name="eps", bufs=1, space="PSUM") as eps, \
             tc.tile_pool(name="dram", bufs=1, space="DRAM") as dram:

            w0 = const.tile([128, 192], F16)
            w1 = const.tile([65, 192], F16)
            ident = const.tile([64, 64], F32)
            corr = const.tile([64, 64], F32)
            nc.scalar.dma_start(w0[:], w0_d[:])
            nc.scalar.dma_start(w1[:], w1_d[:])
            nc.scalar.dma_start(ident[:], ident_d[:])
            nc.scalar.dma_start(corr[:], corr_d[:])
            wqk0, wv0 = w0[:, 0:128], w0[:, 128:192]
            wqk1, wv1 = w1[:, 0:128], w1[:, 128:192]

            # preload the ACT Exp table set off the critical path (~2.7us)
            warm = const.tile([1, 1], F32)
            nc.gpsimd.memset(warm[:], 0.0)
            nc.scalar.activation(warm[:], warm[:], Exp)

            # v, partition-pair packed like out_d (4 MiB fp16)
            v16 = const.tile([128, NHALF // 2], F16)
            # paired energy accumulator: subtile pairs (2s, 2s+1) matmul as
            # lhsT=[q_2s|q_2s+1], rhs=[k_2s|k_2s+1]; the diagonal 64x64
            # blocks accumulate the true energy, off-diagonal is discarded
            ep = eps.tile([128, 128], F32)

            def v_block(vit, vx0h, vx1h):
                for pr in range(NVC // 2):        # chunk pairs
                    vp = vps.tile([128, VC], F32, tag="vp")
                    for h in range(2):
                        vsl = slice((2 * pr + h) * VC, (2 * pr + h + 1) * VC)
                        psl = slice(h * 64, h * 64 + 64)
                        nc.tensor.matmul(vp[psl, :], wv0[:], vx0h[:, vsl],
                                         start=True, stop=False)
                        nc.tensor.matmul(vp[psl, :], wv1[:], vx1h[:, vsl],
                                         start=False, stop=True)
                    blk = vit * (PX // 2) + pr * VC
                    if pr % 2 == 0:
                        nc.scalar.copy(v16[:, blk:blk + VC], vp[:])
                    else:
                        nc.vector.tensor_copy(v16[:, blk:blk + VC], vp[:])

            pending = []
            for it in range(NIT):
                sl = slice(it * PX, (it + 1) * PX)
                x0h = s16.tile([128, PX], F16, tag="x0h")
                nc.sync.dma_start(x0h[:], x01_d[:, sl])
                x1h = s16.tile([65, PX], F16, tag="x1h")
                nc.sync.dma_start(x1h[0:64, :], xb_d[:, sl])
                if it < DEFER + 1:
                    # ones rows live in the round-robin pool slots; later
                    # iterations reuse them untouched
                    nc.gpsimd.memset(x1h[64:65, :], 1.0)

                for grp in range(NSUB // 8):   # 8 proj subtiles per PSUM pair
                    qkp = qkps.tile([128, 1024], F32, tag="qkp")
                    for s8 in range(8):
                        sb = grp * 8 + s8
                        ssl = slice(sb * SUB, (sb + 1) * SUB)
                        osl = slice(s8 * 128, (s8 + 1) * 128)
                        nc.tensor.matmul(qkp[:, osl], x0h[:, ssl], wqk0[:],
                                         start=True, stop=False)
                        nc.tensor.matmul(qkp[:, osl], x1h[:, ssl], wqk1[:],
                                         start=False, stop=True)
                    # scatter-cast: qkp [s][q|k] -> qk_sb [q0..q7 | k0..k7]
                    # so energy subtile-pair operands are contiguous
                    qk_sb = qkpool.tile([128, 1024], F16, tag="qk_sb")
                    dst = qk_sb[:].rearrange("p (qk s b) -> p s qk b",
                                             qk=2, s=8, b=64)
                    srcv = qkp[:].rearrange("p (s qk b) -> p s qk b",
                                            s=8, qk=2, b=64)
                    if grp == 0:
                        nc.scalar.copy(dst, srcv)
                    else:
                        nc.vector.tensor_copy(dst, srcv)
                    for s4 in range(4):
                        first = (it == 0 and grp == 0 and s4 == 0)
                        last = (it == NIT - 1 and grp == NSUB // 8 - 1 and s4 == 3)
                        nc.tensor.matmul(ep[:], qk_sb[:, s4 * 128:s4 * 128 + 128],
                                         qk_sb[:, 512 + s4 * 128:512 + s4 * 128 + 128],
                                         start=first, stop=last)

                if it >= DEFER:
                    v_block(it - DEFER, *pending.pop(0))
                pending.append((x0h, x1h))

            # partial energy -> pairwise AllReduce; the deferred v matmuls
            # below keep PE busy while the collective is in flight
            ep_sb = const.tile([128, 128], F32)
            nc.scalar.copy(ep_sb[:], ep[:])
            ebr = const.tile([64, 64], F32)
            nc.sync.dma_start(ebr[:], ep_sb[64:128, 64:128])
            e_sb = const.tile([64, 64], F32)
            nc.vector.tensor_add(e_sb[:], ep_sb[0:64, 0:64], ebr[:])
            bi = dram.tile([64, 64], F32)
            bo = dram.tile([64, 64], F32)
            nc.sync.dma_start(bi[:], e_sb[:])
            if single_core:
                nc.gpsimd.dma_start(bo[:], bi[:])
            else:
                nc.gpsimd.collective_compute(
                    "AllReduce", mybir.AluOpType.add,
                    replica_groups=[[0, 1], [2, 3], [4, 5], [6, 7]],
                    ins=[bi.opt()], outs=[bo.opt()],
                )

            for d in range(DEFER):
                v_block(NIT - DEFER + d, *pending[d])

            e2 = const.tile([64, 64], F32)
            nc.sync.dma_start(e2[:], bo[:])
            # fp16-weight-truncation correction (see module docstring)
            e2c = const.tile([64, 64], F32)
            nc.vector.tensor_add(e2c[:], e2[:], corr[:])

            # softmax over free dim, scale C^-0.5 = 0.125 folded into exp
            m = const.tile([64, 1], F32)
            nc.vector.reduce_max(m[:], e2c[:], axis=mybir.AxisListType.X)
            mb = const.tile([64, 1], F32)
            nc.vector.tensor_scalar_mul(mb[:], m[:], -0.125)
            attu = const.tile([64, 64], F32)
            s = const.tile([64, 1], F32)
            nc.scalar.activation(attu[:], e2c[:], Exp, bias=mb[:], scale=0.125,
                                 accum_out=s[:])
            r = const.tile([64, 1], F32)
            nc.vector.reciprocal(r[:], s[:])
            att = const.tile([64, 64], F32)
            nc.vector.tensor_scalar_mul(att[:], attu[:], r[:])

            # att^T (PE transpose), cast fp16, stacked at partitions 0 and 64
            # to match the packed v16 halves
            atp = vps.tile([64, 64], F32, tag="vp")
            nc.tensor.transpose(atp[:], att[:], ident[:])
            att16 = const.tile([128, 64], F16)
            nc.scalar.copy(att16[0:64, :], atp[:])
            nc.scalar.copy(att16[64:128, :], atp[:])

            # out = att @ v over chunk pairs: 3 PSUM slots (2 vp + 1 extra
            # bank), one copy per pair alternating ACT/DVE, DMA per 2048px
            out_sb = None
            for pr in range(NHALF // VC // 2):
                if pr % 3 < 2:
                    op = vps.tile([128, VC], F32, tag="vp")
                else:
                    op = eps.tile([128, VC], F32, tag="op2")
                for h in range(2):
                    psl = slice(h * 64, h * 64 + 64)
                    nc.tensor.matmul(op[psl, :], att16[psl, :],
                                     v16[psl, pr * VC:(pr + 1) * VC],
                                     start=True, stop=True)
                w2 = pr % 2
                if w2 == 0:
                    out_sb = outp.tile([128, PX // 2], F16, tag="out_sb")
                dst = out_sb[:, w2 * VC:(w2 + 1) * VC]
                if pr % 2 == 0:
                    nc.scalar.copy(dst, op[:])
                else:
                    nc.vector.tensor_copy(dst, op[:])
                if w2 == 1:
                    g = pr // 2
                    nc.sync.dma_start(
                        out_d[:, g * (PX // 2):(g + 1) * (PX // 2)], out_sb[:])

    nc.compile()
    return nc


def _get_nc():
    if 'nc' not in _CACHE:
        _CACHE['nc'] = _build_bass()
    return _CACHE['nc']


def kernel(rgb, hsv, lab, Wq, bq, Wk, bk, Wv, bv):
    from concourse.bass_utils import run_bass_kernel_spmd

    nc = _get_nc()

    rgb = np.asarray(rgb, dtype=np.float32)
    hsv = np.asarray(hsv, dtype=np.float32)
    lab = np.asarray(lab, dtype=np.float32)
    Wq = np.asarray(Wq, dtype=np.float32)
    Wk = np.asarray(Wk, dtype=np.float32)
    Wv = np.asarray(Wv, dtype=np.float32)
    bq = np.asarray(bq, dtype=np.float32)
    bk = np.asarray(bk, dtype=np.float32)
    bv = np.asarray(bv, dtype=np.float32)

    # weight prep: [192ch + ones-row, outs] with bias row appended, fp16
    wqk = np.concatenate([Wq.T, Wk.T], axis=1)          # [192, 128]
    bqk = np.concatenate([bq, bk])                      # [128]
    wqk_aug = np.vstack([wqk, bqk[None, :]])            # [193, 128]
    wv_aug = np.vstack([Wv.T, bv[None, :]])             # [193, 64]
    wqk16 = wqk_aug.astype(np.float16)
    wv16 = wv_aug.astype(np.float16)

    # energy correction for the coherent fp16 weight-truncation error:
    # E_true - E_hw ~= N*(dWq^T Wk16 + Wq16^T dWk + dWq^T dWk)
    wq64 = wqk_aug[:, 0:64].astype(np.float64)
    wk64 = wqk_aug[:, 64:128].astype(np.float64)
    wq16_64 = wqk16[:, 0:64].astype(np.float64)
    wk16_64 = wqk16[:, 64:128].astype(np.float64)
    dq = wq64 - wq16_64
    dk = wk64 - wk16_64
    corr = (N * (dq.T @ wk16_64 + wq16_64.T @ dk + dq.T @ dk)).astype(np.float32)

    shared = {
        "w0": np.ascontiguousarray(
            np.concatenate([wqk16[0:128], wv16[0:128]], axis=1)),
        "w1": np.ascontiguousarray(
            np.concatenate([wqk16[128:193], wv16[128:193]], axis=1)),
        "ident": np.eye(64, dtype=np.float32),
        "corr": corr,
    }

    in_maps = []
    for c in range(NCORES):
        b, half = c // 2, c % 2
        hs = slice(half * (H // 2), (half + 1) * (H // 2))
        x01 = np.concatenate([rgb[b, :, hs, :].reshape(C, NHALF),
                              hsv[b, :, hs, :].reshape(C, NHALF)], axis=0)
        in_maps.append({
            "x01": np.ascontiguousarray(x01.astype(np.float16)),
            "x_lab": np.ascontiguousarray(
                lab[b, :, hs, :].reshape(C, NHALF).astype(np.float16)),
            **shared,
        })

    res = run_bass_kernel_spmd(nc, in_maps, core_ids=list(range(NCORES)),
                               **_CACHE.get('run_kwargs', {}))
    _CACHE['last_results'] = res
    _CACHE['last_in_maps'] = in_maps

    out = np.empty((B, C, H, W), dtype=np.float32)
    for c in range(NCORES):
        b, half = c // 2, c % 2
        hs = slice(half * (H // 2), (half + 1) * (H // 2))
        # unpack [128, NHALF/2]: part half*64+ch, col P*512+j -> ch, P*1024+half*512+j
        r = res.results[c]["out"].astype(np.float32).reshape(
            2, 64, NHALF // 1024, 512)
        out[b, :, hs, :] = np.transpose(r, (1, 2, 0, 3)).reshape(C, H // 2, W)
    return out
